# revision 26
# baseline (speedup 1.0000x reference)
"""Multi-sense skip-gram (MSSG) loss kernel for Trainium2.

Data-parallel over batch across 8 cores; tables packed row-wise into one
[50000, 2100] bf16 table: row v = [global(300) | emb senses(900) | disamb
senses(900)].

Structure (per 128-element tile, 4 tiles/core):
- 16 single-index indirect gathers with flat 2D SBUF dests. Multi-index
  gathers, 3D unit-dim dest views, and SBUF->SBUF accum_op DMAs all pass
  CoreSim but CRASH real trn2 HW (re-verified) - do not reintroduce.
- Engine split: DVE owns products / fold trees / reduces (bf16, 2x mode);
  ACT owns all transcendentals and the 60 per-(c,s) weighted-sum scale
  multiplies (activation-Copy with [P,1] scale APs) - ACT segment-reduce
  (Copy+accum_out per 300-wide segment, ~0.8us each) was measured 4.7x
  more expensive than DVE fold trees and is NOT used.
- Scheduler control: the serial disamb chain (z dots -> softmax -> scale
  -> fold -> ctx_out) keeps natural priority; off-path work (pos/neg
  products+folds, sigmoid mixing, per-tile products) is demoted by a
  large priority offset so the Tile scheduler slots it into chain stalls
  instead of ahead of the chain (v2 lost ~20us to such inversions).
- Loss tail: per-tile mixture factors are multiplied (DVE mult-reduce)
  and ONE Ln runs at the end: sum(log(w)) == log(prod(w)); avoids Exp/Ln
  activation-table thrash (14 reloads -> 2).
- Softmax/sigmoid reciprocals via reciprocal_approx_fast (~51 ULP,
  inputs bounded away from 0); z/logit reduces output bf16 (values are
  O(1e-4) dots; bf16 keeps 2x DVE mode).
"""

import numpy as np

NUM_SENSE = 3
EMB_DIM = 300
VOCAB = 50000
BATCH = 4096
CTX = 10
NEG = 5
N_CORES = 8
P = 128
PER_CORE = BATCH // N_CORES  # 512
TILES = PER_CORE // P        # 4
D = EMB_DIM
CS = CTX * NUM_SENSE         # 30
SN = NUM_SENSE * NEG         # 15
PN = CS + SN                 # 45
SEG = CTX + NEG              # 15 (per-sense segment: 10 pos + 5 neg)
RowLen = D + 2 * NUM_SENSE * D  # 2100: [glob | emb | dis]
EMB_OFF = D                  # 300
DIS_OFF = D + NUM_SENSE * D  # 1200

LOWPRI = 1_000_000
ACT_SCALES = 15              # wsum scale-mults per disamb step on ACT (of 30)
_CACHE = {}


def _build_bass():
    key = "nc"
    if key in _CACHE:
        return _CACHE[key]

    from contextlib import contextmanager

    import concourse.bass as bass
    import concourse.bacc as bacc
    import concourse.tile as tile
    from concourse import mybir

    F32 = mybir.dt.float32
    BF16 = mybir.dt.bfloat16
    FP8 = mybir.dt.float8e4
    I32 = mybir.dt.int32
    AX = mybir.AxisListType
    OP = mybir.AluOpType
    AF = mybir.ActivationFunctionType
    TINY = float(np.finfo(np.float32).tiny)

    nc = bacc.Bacc("TRN2", target_bir_lowering=False, debug=False)

    # bf16 table. (fp8 with cast-during-gather was tried and is correct,
    # but the in-flight cast costs the SWDGE/SDMA path ~+160ns per gather
    # while DMA bandwidth is not the bottleneck - net regression.)
    packed = nc.dram_tensor("packed", [VOCAB, RowLen], BF16, kind="ExternalInput")
    # idx columns: 0..9 ctx, 10 word, 11..15 neg
    idx = nc.dram_tensor("idx", [PER_CORE, 16], I32, kind="ExternalInput")
    out_d = nc.dram_tensor("out", [1, 1], F32, kind="ExternalOutput")

    def tt(out, a, b, op=OP.add):
        nc.vector.tensor_tensor(out=out, in0=a, in1=b, op=op)

    with tile.TileContext(nc) as tc:

        @contextmanager
        def lowpri(off=LOWPRI):
            # mirror tc.high_priority's mechanism with a positive offset:
            # demoted instructions look "later" to the Tile scheduler, so
            # they fill engine idle slots instead of delaying the chain.
            tc.cur_priority += off
            try:
                yield
            finally:
                tc.cur_priority -= off

        with (
            tc.tile_pool(name="gather", bufs=2) as gp,
            tc.tile_pool(name="tmpp", bufs=2) as tp,
            tc.tile_pool(name="small", bufs=2) as sp,
            tc.tile_pool(name="persist", bufs=1) as pp,
            tc.tile_pool(name="psum", bufs=1, space="PSUM") as psp,
        ):
            ones = pp.tile([P, 1], F32)
            PRODS = pp.tile([P, 2 * TILES], F32)
            nc.vector.memset(ones[:], 1.0)

            for t in range(TILES):
                rows = slice(t * P, (t + 1) * P)
                ix = gp.tile([P, 16], I32)
                nc.sync.dma_start(out=ix[:], in_=idx[rows, :])

                PKc = gp.tile([P, CTX * RowLen], BF16)  # 10 ctx rows
                PKw = gp.tile([P, RowLen], BF16)        # word row
                NG = gp.tile([P, NEG * D], BF16)        # neg: glob part only
                PK3 = PKc[:].rearrange("p (k x) -> p k x", x=RowLen)
                NG3 = NG[:].rearrange("p (n d) -> p n d", d=D)

                def gather1(dst, offs):
                    nc.gpsimd.indirect_dma_start(
                        out=dst, out_offset=None, in_=packed[:],
                        in_offset=bass.IndirectOffsetOnAxis(ap=offs, axis=0),
                    )

                # ctx rows first (critical chain), then word, then negs
                for k in range(CTX):
                    gather1(PKc[:, k * RowLen:(k + 1) * RowLen], ix[:, k:k + 1])
                gather1(PKw[:, :], ix[:, 10:11])
                for k in range(NEG):
                    gather1(NG[:, k * D:(k + 1) * D], ix[:, 11 + k:12 + k])

                CT3 = PK3[:, :, 0:D]                                   # [P,10,300]
                AS4 = PK3[:, :, EMB_OFF:DIS_OFF].rearrange(
                    "p c (s d) -> p c s d", d=D)                       # [P,10,3,300]
                AD4 = PK3[:, :, DIS_OFF:RowLen].rearrange(
                    "p c (s d) -> p c s d", d=D)
                SEN3 = PKw[:, EMB_OFF:DIS_OFF].rearrange(
                    "p (s d) -> p s d", d=D)                           # [P,3,300]
                DIS3 = PKw[:, DIS_OFF:RowLen].rearrange(
                    "p (s d) -> p s d", d=D)

                TMP = tp.tile([P, CS * D], BF16, tag="tmp_d")
                T3 = TMP[:].rearrange("p (k d) -> p k d", d=D)
                TMPQ = tp.tile([P, PN * D], BF16, tag="tmp_pn")
                Q3 = TMPQ[:].rearrange("p (k d) -> p k d", d=D)

                # ---- ctx1 = sum_c CT (mean deferred via exp scale) ----
                # computed on the Pool engine, right after its own gathers:
                # it's off DVE's critical stream and effectively prefetched
                # for tiles > 0. First fold split per (c, c+5) pair so each
                # starts as its two gathers land.
                c1a = sp.tile([P, 5 * D], BF16)
                c1b = sp.tile([P, 2 * D], BF16)
                ctx1 = sp.tile([P, D], BF16)
                c1a3 = c1a[:].rearrange("p (c d) -> p c d", d=D)

                def ptt(out, a, b):
                    nc.gpsimd.tensor_tensor(out=out, in0=a, in1=b, op=OP.add)

                for c in range(5):
                    ptt(c1a3[:, c:c + 1, :], CT3[:, c:c + 1, :],
                        CT3[:, c + 5:c + 6, :])
                ptt(c1b[:], c1a[:, 0:2 * D], c1a[:, 2 * D:4 * D])
                ptt(c1b[:, 0:D], c1b[:, 0:D], c1b[:, D:2 * D])
                ptt(ctx1[:], c1b[:, 0:D], c1a[:, 4 * D:5 * D])

                def disamb_step(ctx_vec, ctx_out, si):
                    # dot products z[c,s] = AD[c,s] . ctx  (products in two
                    # halves so folds overlap the second multiply)
                    T4 = T3.rearrange("p (c s) d -> p c s d", s=NUM_SENSE)
                    cb = ctx_vec[:].unsqueeze(1).unsqueeze(1) \
                        .to_broadcast([P, 5, NUM_SENSE, D])
                    tt(T4[:, 0:5], AD4[:, 0:5], cb, OP.mult)
                    tt(T4[:, 5:10], AD4[:, 5:10], cb, OP.mult)
                    z = sp.tile([P, CS], BF16, tag=f"z{si}")
                    tt(T3[:, 0:15, 0:150], T3[:, 0:15, 0:150],
                       T3[:, 0:15, 150:300])
                    tt(T3[:, 15:30, 0:150], T3[:, 15:30, 0:150],
                       T3[:, 15:30, 150:300])
                    tt(T3[:, :, 0:74], T3[:, :, 0:74], T3[:, :, 76:150])
                    tt(T3[:, :, 0:38], T3[:, :, 0:38], T3[:, :, 38:76])
                    with nc.allow_low_precision("z dots are O(1e-4); bf16 "
                                                "keeps DVE 2x mode"):
                        nc.vector.tensor_reduce(
                            out=z[:], in_=T3[:, :, 0:38], axis=AX.X, op=OP.add)
                    # softmax over s (ACT exp, DVE small ops)
                    E = sp.tile([P, CS], F32, tag=f"E{si}")
                    nc.scalar.activation(out=E[:], in_=z[:], func=AF.Exp,
                                         scale=1.0 / CTX)
                    S = sp.tile([P, CTX], F32, tag=f"S{si}")
                    nc.vector.tensor_reduce(
                        out=S[:], in_=E[:].rearrange("p (c s) -> p c s",
                                                     s=NUM_SENSE),
                        axis=AX.X, op=OP.add)
                    R = sp.tile([P, CTX], F32, tag=f"R{si}")
                    nc.vector.reciprocal_approx_fast(R[:], S[:])
                    AL = sp.tile([P, CS], F32, tag=f"AL{si}")
                    tt(AL[:].rearrange("p (c s) -> p c s", s=NUM_SENSE),
                       E[:].rearrange("p (c s) -> p c s", s=NUM_SENSE),
                       R[:].unsqueeze(2).to_broadcast([P, CTX, NUM_SENSE]),
                       OP.mult)
                    # weighted sum: scale-mults split ACT (rows < ACT_SCALES)
                    # / DVE tensor_scalar 4x (rest) so the chain's scale
                    # phase is ~max(9.3, 6.6)us, not 18.6; folded on DVE in
                    # two 15-row subtrees as the halves complete.
                    for k in range(ACT_SCALES):
                        nc.scalar.mul(
                            T3[:, k, :],
                            AS4[:, k // NUM_SENSE, k % NUM_SENSE, :],
                            AL[:, k:k + 1])
                    for k in range(ACT_SCALES, CS):
                        nc.vector.tensor_scalar_mul(
                            out=T3[:, k, :],
                            in0=AS4[:, k // NUM_SENSE, k % NUM_SENSE, :],
                            scalar1=AL[:, k:k + 1])
                    for b in (15, 0):  # DVE-scaled half folds first
                        tt(T3[:, b:b + 7, :], T3[:, b:b + 7, :],
                           T3[:, b + 7:b + 14, :])
                        tt(T3[:, b:b + 3, :], T3[:, b:b + 3, :],
                           T3[:, b + 3:b + 6, :])
                        tt(T3[:, b:b + 1, :], T3[:, b:b + 1, :],
                           T3[:, b + 1:b + 2, :])
                        tt(T3[:, b:b + 1, :], T3[:, b:b + 1, :],
                           T3[:, b + 2:b + 3, :])
                        tt(T3[:, b:b + 1, :], T3[:, b:b + 1, :],
                           T3[:, b + 6:b + 7, :])
                        tt(T3[:, b:b + 1, :], T3[:, b:b + 1, :],
                           T3[:, b + 14:b + 15, :])
                    tt(ctx_out[:].unsqueeze(1), T3[:, 0:1, :], T3[:, 15:16, :])

                ctx2 = sp.tile([P, D], BF16)
                ctx3 = sp.tile([P, D], BF16)
                disamb_step(ctx1, ctx2, 0)

                # ---- fillers: pos/neg logit products (demoted priority) ----
                # Q3 rows s-major-15: [s*15 .. s*15+10) = pos(c), then neg(n)
                toff = LOWPRI if t < TILES - 1 else 2000
                with lowpri(toff):
                    Q4 = Q3.rearrange("p (s g) d -> p s g d", g=SEG)
                    tt(Q4[:, :, 0:CTX, :],
                       CT3.unsqueeze(1).to_broadcast([P, NUM_SENSE, CTX, D]),
                       SEN3.unsqueeze(2).to_broadcast([P, NUM_SENSE, CTX, D]),
                       OP.mult)
                    tt(Q4[:, :, CTX:SEG, :],
                       NG3.unsqueeze(1).to_broadcast([P, NUM_SENSE, NEG, D]),
                       SEN3.unsqueeze(2).to_broadcast([P, NUM_SENSE, NEG, D]),
                       OP.mult)

                disamb_step(ctx2, ctx3, 1)

                # ---- alpha = softmax_s(DIS . ctx3 / CTX) ----
                tt(T3[:, 0:NUM_SENSE, :], DIS3,
                   ctx3[:].unsqueeze(1).to_broadcast([P, NUM_SENSE, D]),
                   OP.mult)
                av = sp.tile([P, NUM_SENSE], BF16, tag="av")
                a3 = T3[:, 0:NUM_SENSE, :]
                tt(a3[:, :, 0:150], a3[:, :, 0:150], a3[:, :, 150:300])
                tt(a3[:, :, 0:74], a3[:, :, 0:74], a3[:, :, 76:150])
                tt(a3[:, :, 0:38], a3[:, :, 0:38], a3[:, :, 38:76])
                with nc.allow_low_precision("alpha dots are O(1e-4)"):
                    nc.vector.tensor_reduce(
                        out=av[:], in_=a3[:, :, 0:38], axis=AX.X, op=OP.add)
                Ea = sp.tile([P, NUM_SENSE], F32, tag="Ea")
                nc.scalar.activation(out=Ea[:], in_=av[:], func=AF.Exp,
                                     scale=1.0 / CTX)
                Sa = sp.tile([P, 1], F32, tag="Sa")
                nc.vector.tensor_reduce(
                    out=Sa[:], in_=Ea[:], axis=AX.X, op=OP.add)
                Ra = sp.tile([P, 1], F32, tag="Ra")
                nc.vector.reciprocal_approx_fast(Ra[:], Sa[:])
                ALS = sp.tile([P, NUM_SENSE], F32, tag="ALS")
                nc.vector.tensor_scalar_mul(out=ALS[:], in0=Ea[:],
                                            scalar1=Ra[:, 0:1])

                # ---- demoted tail: segment-reduce pos/neg logits on ACT
                # (Copy+accum_out; keeps ACT loaded while DVE owns the chain)
                with lowpri(toff):
                    PLN = sp.tile([P, PN], F32, tag="PLN")
                    SCR = [sp.tile([P, D], BF16, tag=f"scr{i}",
                                   name=f"scr{i}")
                           for i in range(4)]
                    for k in range(PN):
                        nc.scalar.activation(
                            out=SCR[k % 4][:], in_=Q3[:, k, :],
                            func=AF.Copy, accum_out=PLN[:, k:k + 1])
                    # sigmoid 1/(1+exp(-x))
                    PE_ = sp.tile([P, PN], F32, tag="sigE")
                    nc.scalar.activation(out=PE_[:], in_=PLN[:], func=AF.Exp,
                                         scale=-1.0)
                    nc.vector.tensor_scalar_add(out=PE_[:], in0=PE_[:],
                                                scalar1=1.0)
                    PPr = sp.tile([P, PN], F32, tag="sigR")
                    nc.vector.reciprocal_approx_fast(PPr[:], PE_[:])
                    # mix over senses: M[P,15] = sum_s ALS_s * PPr[s-seg]
                    M = sp.tile([P, SEG], F32, tag="M")
                    M2 = sp.tile([P, 2 * SEG], F32, tag="M2")
                    nc.scalar.mul(M[:], PPr[:, 0:SEG], ALS[:, 0:1])
                    nc.scalar.mul(M2[:, 0:SEG], PPr[:, SEG:2 * SEG],
                                  ALS[:, 1:2])
                    nc.scalar.mul(M2[:, SEG:2 * SEG], PPr[:, 2 * SEG:3 * SEG],
                                  ALS[:, 2:3])
                    tt(M2[:, 0:SEG], M2[:, 0:SEG], M2[:, SEG:2 * SEG])
                    tt(M[:], M[:], M2[:, 0:SEG])
                    # V = 1 - Wn for the neg slots (on ACT: Copy(-x+1))
                    V = sp.tile([P, NEG], F32, tag="V")
                    nc.scalar.activation(out=V[:], in_=M[:, CTX:SEG],
                                         func=AF.Copy, scale=-1.0, bias=1.0)
                    nc.vector.tensor_reduce(
                        out=PRODS[:, 2 * t:2 * t + 1], in_=M[:, 0:CTX],
                        axis=AX.X, op=OP.mult)
                    nc.vector.tensor_reduce(
                        out=PRODS[:, 2 * t + 1:2 * t + 2], in_=V[:],
                        axis=AX.X, op=OP.mult)

            # ---- loss tail: ln(prod of all factors), summed over partitions
            PR = pp.tile([P, 1], F32)
            nc.vector.tensor_reduce(out=PR[:], in_=PRODS[:], axis=AX.X,
                                    op=OP.mult)
            nc.vector.tensor_scalar_max(out=PR[:], in0=PR[:], scalar1=TINY)
            LNP = pp.tile([P, 1], F32)
            nc.scalar.activation(out=LNP[:], in_=PR[:], func=AF.Ln)
            ps = psp.tile([1, 1], F32)
            nc.tensor.matmul(out=ps[:], lhsT=LNP[:], rhs=ones[:],
                             start=True, stop=True)
            res = pp.tile([1, 1], F32)
            nc.scalar.copy(res[:], ps[:])
            nc.sync.dma_start(out=out_d[:, :], in_=res[:])

    nc.compile()
    _CACHE[key] = nc
    return nc


def _prep_inputs(word_ids, context_ids, neg_ids,
                 emb_weight, global_emb_weight, disamb_weight):
    import ml_dtypes
    BF = ml_dtypes.bfloat16
    word_ids = np.asarray(word_ids).astype(np.int32).reshape(BATCH)
    context_ids = np.asarray(context_ids).astype(np.int32).reshape(BATCH, CTX)
    neg_ids = np.asarray(neg_ids).astype(np.int32).reshape(BATCH, NEG)
    packed = np.empty((VOCAB, RowLen), dtype=BF)
    packed[:, 0:D] = np.asarray(
        global_emb_weight, dtype=np.float32).astype(BF).reshape(VOCAB, D)
    packed[:, EMB_OFF:DIS_OFF] = np.asarray(
        emb_weight, dtype=np.float32).astype(BF).reshape(VOCAB, NUM_SENSE * D)
    packed[:, DIS_OFF:RowLen] = np.asarray(
        disamb_weight, dtype=np.float32).astype(BF).reshape(VOCAB, NUM_SENSE * D)
    # idx columns: 0..9 ctx, 10 word, 11..15 neg
    idx_all = np.concatenate(
        [context_ids, word_ids[:, None], neg_ids], axis=1).astype(np.int32)
    in_maps = []
    for c in range(N_CORES):
        sl = slice(c * PER_CORE, (c + 1) * PER_CORE)
        in_maps.append({
            "packed": packed,
            "idx": np.ascontiguousarray(idx_all[sl]),
        })
    return in_maps


def kernel(word_ids, context_ids, context_masks, neg_ids,
           emb_weight, global_emb_weight, disamb_weight):
    from concourse import bass_utils
    nc = _build_bass()
    in_maps = _prep_inputs(word_ids, context_ids, neg_ids,
                           emb_weight, global_emb_weight, disamb_weight)
    res = bass_utils.run_bass_kernel_spmd(nc, in_maps, core_ids=list(range(N_CORES)))
    total = 0.0
    for r in res.results:
        total += float(np.asarray(r["out"]).reshape(-1)[0])
    loss = -total / float(BATCH * CTX)
    return np.array(loss, dtype=np.float32)


# revision 29
# speedup vs baseline: 1.0032x; 1.0032x over previous
"""Multi-sense skip-gram (MSSG) loss kernel for Trainium2.

Data-parallel over batch across 8 cores; tables packed row-wise into one
[50000, 2100] bf16 table: row v = [global(300) | emb senses(900) | disamb
senses(900)].

Structure (per 128-element tile, 4 tiles/core):
- 16 single-index indirect gathers with flat 2D SBUF dests. Multi-index
  gathers, 3D unit-dim dest views, and SBUF->SBUF accum_op DMAs all pass
  CoreSim but CRASH real trn2 HW (re-verified) - do not reintroduce.
- Engine split: DVE owns products / fold trees / reduces (bf16, 2x mode);
  ACT owns all transcendentals and the 60 per-(c,s) weighted-sum scale
  multiplies (activation-Copy with [P,1] scale APs) - ACT segment-reduce
  (Copy+accum_out per 300-wide segment, ~0.8us each) was measured 4.7x
  more expensive than DVE fold trees and is NOT used.
- Scheduler control: the serial disamb chain (z dots -> softmax -> scale
  -> fold -> ctx_out) keeps natural priority; off-path work (pos/neg
  products+folds, sigmoid mixing, per-tile products) is demoted by a
  large priority offset so the Tile scheduler slots it into chain stalls
  instead of ahead of the chain (v2 lost ~20us to such inversions).
- Loss tail: per-tile mixture factors are multiplied (DVE mult-reduce)
  and ONE Ln runs at the end: sum(log(w)) == log(prod(w)); avoids Exp/Ln
  activation-table thrash (14 reloads -> 2).
- Softmax/sigmoid reciprocals via reciprocal_approx_fast (~51 ULP,
  inputs bounded away from 0); z/logit reduces output bf16 (values are
  O(1e-4) dots; bf16 keeps 2x DVE mode).
"""

import numpy as np

NUM_SENSE = 3
EMB_DIM = 300
VOCAB = 50000
BATCH = 4096
CTX = 10
NEG = 5
N_CORES = 8
P = 128
PER_CORE = BATCH // N_CORES  # 512
TILES = PER_CORE // P        # 4
D = EMB_DIM
CS = CTX * NUM_SENSE         # 30
SN = NUM_SENSE * NEG         # 15
PN = CS + SN                 # 45
SEG = CTX + NEG              # 15 (per-sense segment: 10 pos + 5 neg)
RowLen = D + 2 * NUM_SENSE * D  # 2100: [glob | emb | dis]
EMB_OFF = D                  # 300
DIS_OFF = D + NUM_SENSE * D  # 1200

LOWPRI = 1_000_000
ACT_SCALES = 15              # wsum scale-mults per disamb step on ACT (of 30)
_CACHE = {}


def _build_bass():
    key = "nc"
    if key in _CACHE:
        return _CACHE[key]

    from contextlib import contextmanager

    import concourse.bass as bass
    import concourse.bacc as bacc
    import concourse.tile as tile
    from concourse import mybir

    F32 = mybir.dt.float32
    BF16 = mybir.dt.bfloat16
    FP8 = mybir.dt.float8e4
    I32 = mybir.dt.int32
    AX = mybir.AxisListType
    OP = mybir.AluOpType
    AF = mybir.ActivationFunctionType
    TINY = float(np.finfo(np.float32).tiny)

    nc = bacc.Bacc("TRN2", target_bir_lowering=False, debug=False)

    # bf16 table. (fp8 with cast-during-gather was tried and is correct,
    # but the in-flight cast costs the SWDGE/SDMA path ~+160ns per gather
    # while DMA bandwidth is not the bottleneck - net regression.)
    packed = nc.dram_tensor("packed", [VOCAB, RowLen], BF16, kind="ExternalInput")
    # idx columns: 0..9 ctx, 10 word, 11..15 neg
    idx = nc.dram_tensor("idx", [PER_CORE, 16], I32, kind="ExternalInput")
    out_d = nc.dram_tensor("out", [1, 1], F32, kind="ExternalOutput")

    def tt(out, a, b, op=OP.add):
        nc.vector.tensor_tensor(out=out, in0=a, in1=b, op=op)

    with tile.TileContext(nc) as tc:

        @contextmanager
        def lowpri(off=LOWPRI):
            # mirror tc.high_priority's mechanism with a positive offset:
            # demoted instructions look "later" to the Tile scheduler, so
            # they fill engine idle slots instead of delaying the chain.
            tc.cur_priority += off
            try:
                yield
            finally:
                tc.cur_priority -= off

        with (
            tc.tile_pool(name="gather", bufs=2) as gp,
            tc.tile_pool(name="tmpp", bufs=2) as tp,
            tc.tile_pool(name="small", bufs=2) as sp,
            tc.tile_pool(name="persist", bufs=1) as pp,
            tc.tile_pool(name="psum", bufs=1, space="PSUM") as psp,
        ):
            ones = pp.tile([P, 1], F32)
            PRODS = pp.tile([P, 2 * TILES], F32)
            nc.vector.memset(ones[:], 1.0)

            for t in range(TILES):
                rows = slice(t * P, (t + 1) * P)
                ix = gp.tile([P, 16], I32)
                nc.sync.dma_start(out=ix[:], in_=idx[rows, :])

                PKc = gp.tile([P, CTX * RowLen], BF16)  # 10 ctx rows
                PKw = gp.tile([P, RowLen], BF16)        # word row
                NG = gp.tile([P, NEG * D], BF16)        # neg: glob part only
                PK3 = PKc[:].rearrange("p (k x) -> p k x", x=RowLen)
                NG3 = NG[:].rearrange("p (n d) -> p n d", d=D)

                def gather1(dst, offs):
                    nc.gpsimd.indirect_dma_start(
                        out=dst, out_offset=None, in_=packed[:],
                        in_offset=bass.IndirectOffsetOnAxis(ap=offs, axis=0),
                    )

                # ctx rows first (critical chain), then word, then negs
                for k in range(CTX):
                    gather1(PKc[:, k * RowLen:(k + 1) * RowLen], ix[:, k:k + 1])
                gather1(PKw[:, :], ix[:, 10:11])
                for k in range(NEG):
                    gather1(NG[:, k * D:(k + 1) * D], ix[:, 11 + k:12 + k])

                # Calibrate the Tile scheduler's optimistic SWDGE model: a
                # gather really takes ~1.85us on the Pool stream (ring-
                # capacity stalls the model does not see), so tile t's
                # consumers cannot start before ~1.85*(16t+11)us. Without
                # this floor the scheduler hoists next-tile work into chain
                # slots that are not actually free, head-blocking the
                # in-order engine streams. (Applies to compute only - the
                # gathers above inherit the previous tile's floor.)
                tc.tile_set_cur_wait((4000 + 1850 * (16 * t + 11)) / 1e6)

                CT3 = PK3[:, :, 0:D]                                   # [P,10,300]
                AS4 = PK3[:, :, EMB_OFF:DIS_OFF].rearrange(
                    "p c (s d) -> p c s d", d=D)                       # [P,10,3,300]
                AD4 = PK3[:, :, DIS_OFF:RowLen].rearrange(
                    "p c (s d) -> p c s d", d=D)
                SEN3 = PKw[:, EMB_OFF:DIS_OFF].rearrange(
                    "p (s d) -> p s d", d=D)                           # [P,3,300]
                DIS3 = PKw[:, DIS_OFF:RowLen].rearrange(
                    "p (s d) -> p s d", d=D)

                TMP = tp.tile([P, CS * D], BF16, tag="tmp_d")
                T3 = TMP[:].rearrange("p (k d) -> p k d", d=D)
                TMPQ = tp.tile([P, PN * D], BF16, tag="tmp_pn")
                Q3 = TMPQ[:].rearrange("p (k d) -> p k d", d=D)

                # ---- ctx1 = sum_c CT (mean deferred via exp scale) ----
                # computed on the Pool engine, right after its own gathers:
                # it's off DVE's critical stream and effectively prefetched
                # for tiles > 0. First fold split per (c, c+5) pair so each
                # starts as its two gathers land.
                c1a = sp.tile([P, 5 * D], BF16)
                c1b = sp.tile([P, 2 * D], BF16)
                ctx1 = sp.tile([P, D], BF16)
                c1a3 = c1a[:].rearrange("p (c d) -> p c d", d=D)

                def ptt(out, a, b):
                    nc.gpsimd.tensor_tensor(out=out, in0=a, in1=b, op=OP.add)

                for c in range(5):
                    ptt(c1a3[:, c:c + 1, :], CT3[:, c:c + 1, :],
                        CT3[:, c + 5:c + 6, :])
                ptt(c1b[:], c1a[:, 0:2 * D], c1a[:, 2 * D:4 * D])
                ptt(c1b[:, 0:D], c1b[:, 0:D], c1b[:, D:2 * D])
                ptt(ctx1[:], c1b[:, 0:D], c1a[:, 4 * D:5 * D])

                def disamb_step(ctx_vec, ctx_out, si):
                    # dot products z[c,s] = AD[c,s] . ctx  (products in two
                    # halves so folds overlap the second multiply)
                    T4 = T3.rearrange("p (c s) d -> p c s d", s=NUM_SENSE)
                    cb = ctx_vec[:].unsqueeze(1).unsqueeze(1) \
                        .to_broadcast([P, 5, NUM_SENSE, D])
                    tt(T4[:, 0:5], AD4[:, 0:5], cb, OP.mult)
                    tt(T4[:, 5:10], AD4[:, 5:10], cb, OP.mult)
                    z = sp.tile([P, CS], BF16, tag=f"z{si}")
                    tt(T3[:, 0:15, 0:150], T3[:, 0:15, 0:150],
                       T3[:, 0:15, 150:300])
                    tt(T3[:, 15:30, 0:150], T3[:, 15:30, 0:150],
                       T3[:, 15:30, 150:300])
                    tt(T3[:, :, 0:74], T3[:, :, 0:74], T3[:, :, 76:150])
                    tt(T3[:, :, 0:38], T3[:, :, 0:38], T3[:, :, 38:76])
                    with nc.allow_low_precision("z dots are O(1e-4); bf16 "
                                                "keeps DVE 2x mode"):
                        nc.vector.tensor_reduce(
                            out=z[:], in_=T3[:, :, 0:38], axis=AX.X, op=OP.add)
                    # softmax over s (ACT exp, DVE small ops)
                    E = sp.tile([P, CS], F32, tag=f"E{si}")
                    nc.scalar.activation(out=E[:], in_=z[:], func=AF.Exp,
                                         scale=1.0 / CTX)
                    S = sp.tile([P, CTX], F32, tag=f"S{si}")
                    nc.vector.tensor_reduce(
                        out=S[:], in_=E[:].rearrange("p (c s) -> p c s",
                                                     s=NUM_SENSE),
                        axis=AX.X, op=OP.add)
                    R = sp.tile([P, CTX], F32, tag=f"R{si}")
                    nc.vector.reciprocal_approx_fast(R[:], S[:])
                    AL = sp.tile([P, CS], F32, tag=f"AL{si}")
                    tt(AL[:].rearrange("p (c s) -> p c s", s=NUM_SENSE),
                       E[:].rearrange("p (c s) -> p c s", s=NUM_SENSE),
                       R[:].unsqueeze(2).to_broadcast([P, CTX, NUM_SENSE]),
                       OP.mult)
                    # weighted sum: scale-mults split ACT (rows < ACT_SCALES)
                    # / DVE tensor_scalar 4x (rest) so the chain's scale
                    # phase is ~max(9.3, 6.6)us, not 18.6; folded on DVE in
                    # two 15-row subtrees as the halves complete.
                    for k in range(ACT_SCALES):
                        nc.scalar.mul(
                            T3[:, k, :],
                            AS4[:, k // NUM_SENSE, k % NUM_SENSE, :],
                            AL[:, k:k + 1])
                    for k in range(ACT_SCALES, CS):
                        nc.vector.tensor_scalar_mul(
                            out=T3[:, k, :],
                            in0=AS4[:, k // NUM_SENSE, k % NUM_SENSE, :],
                            scalar1=AL[:, k:k + 1])
                    for b in (15, 0):  # DVE-scaled half folds first
                        tt(T3[:, b:b + 7, :], T3[:, b:b + 7, :],
                           T3[:, b + 7:b + 14, :])
                        tt(T3[:, b:b + 3, :], T3[:, b:b + 3, :],
                           T3[:, b + 3:b + 6, :])
                        tt(T3[:, b:b + 1, :], T3[:, b:b + 1, :],
                           T3[:, b + 1:b + 2, :])
                        tt(T3[:, b:b + 1, :], T3[:, b:b + 1, :],
                           T3[:, b + 2:b + 3, :])
                        tt(T3[:, b:b + 1, :], T3[:, b:b + 1, :],
                           T3[:, b + 6:b + 7, :])
                        tt(T3[:, b:b + 1, :], T3[:, b:b + 1, :],
                           T3[:, b + 14:b + 15, :])
                    tt(ctx_out[:].unsqueeze(1), T3[:, 0:1, :], T3[:, 15:16, :])

                ctx2 = sp.tile([P, D], BF16)
                ctx3 = sp.tile([P, D], BF16)
                disamb_step(ctx1, ctx2, 0)

                # ---- fillers: pos/neg logit products (demoted priority) ----
                # Q3 rows s-major-15: [s*15 .. s*15+10) = pos(c), then neg(n)
                toff = LOWPRI if t < TILES - 1 else 2000
                with lowpri(toff):
                    Q4 = Q3.rearrange("p (s g) d -> p s g d", g=SEG)
                    tt(Q4[:, :, 0:CTX, :],
                       CT3.unsqueeze(1).to_broadcast([P, NUM_SENSE, CTX, D]),
                       SEN3.unsqueeze(2).to_broadcast([P, NUM_SENSE, CTX, D]),
                       OP.mult)
                    tt(Q4[:, :, CTX:SEG, :],
                       NG3.unsqueeze(1).to_broadcast([P, NUM_SENSE, NEG, D]),
                       SEN3.unsqueeze(2).to_broadcast([P, NUM_SENSE, NEG, D]),
                       OP.mult)

                disamb_step(ctx2, ctx3, 1)

                # ---- alpha = softmax_s(DIS . ctx3 / CTX) ----
                tt(T3[:, 0:NUM_SENSE, :], DIS3,
                   ctx3[:].unsqueeze(1).to_broadcast([P, NUM_SENSE, D]),
                   OP.mult)
                av = sp.tile([P, NUM_SENSE], BF16, tag="av")
                a3 = T3[:, 0:NUM_SENSE, :]
                tt(a3[:, :, 0:150], a3[:, :, 0:150], a3[:, :, 150:300])
                tt(a3[:, :, 0:74], a3[:, :, 0:74], a3[:, :, 76:150])
                tt(a3[:, :, 0:38], a3[:, :, 0:38], a3[:, :, 38:76])
                with nc.allow_low_precision("alpha dots are O(1e-4)"):
                    nc.vector.tensor_reduce(
                        out=av[:], in_=a3[:, :, 0:38], axis=AX.X, op=OP.add)
                Ea = sp.tile([P, NUM_SENSE], F32, tag="Ea")
                nc.scalar.activation(out=Ea[:], in_=av[:], func=AF.Exp,
                                     scale=1.0 / CTX)
                Sa = sp.tile([P, 1], F32, tag="Sa")
                nc.vector.tensor_reduce(
                    out=Sa[:], in_=Ea[:], axis=AX.X, op=OP.add)
                Ra = sp.tile([P, 1], F32, tag="Ra")
                nc.vector.reciprocal_approx_fast(Ra[:], Sa[:])
                ALS = sp.tile([P, NUM_SENSE], F32, tag="ALS")
                nc.vector.tensor_scalar_mul(out=ALS[:], in0=Ea[:],
                                            scalar1=Ra[:, 0:1])

                # ---- demoted tail: segment-reduce pos/neg logits on ACT
                # (Copy+accum_out; keeps ACT loaded while DVE owns the chain)
                with lowpri(toff):
                    PLN = sp.tile([P, PN], F32, tag="PLN")
                    SCR = [sp.tile([P, D], BF16, tag=f"scr{i}",
                                   name=f"scr{i}")
                           for i in range(4)]
                    for k in range(PN):
                        nc.scalar.activation(
                            out=SCR[k % 4][:], in_=Q3[:, k, :],
                            func=AF.Copy, accum_out=PLN[:, k:k + 1])
                    # sigmoid 1/(1+exp(-x))
                    PE_ = sp.tile([P, PN], F32, tag="sigE")
                    nc.scalar.activation(out=PE_[:], in_=PLN[:], func=AF.Exp,
                                         scale=-1.0)
                    nc.vector.tensor_scalar_add(out=PE_[:], in0=PE_[:],
                                                scalar1=1.0)
                    PPr = sp.tile([P, PN], F32, tag="sigR")
                    nc.vector.reciprocal_approx_fast(PPr[:], PE_[:])
                    # mix over senses: M[P,15] = sum_s ALS_s * PPr[s-seg]
                    M = sp.tile([P, SEG], F32, tag="M")
                    M2 = sp.tile([P, 2 * SEG], F32, tag="M2")
                    nc.scalar.mul(M[:], PPr[:, 0:SEG], ALS[:, 0:1])
                    nc.scalar.mul(M2[:, 0:SEG], PPr[:, SEG:2 * SEG],
                                  ALS[:, 1:2])
                    nc.scalar.mul(M2[:, SEG:2 * SEG], PPr[:, 2 * SEG:3 * SEG],
                                  ALS[:, 2:3])
                    tt(M2[:, 0:SEG], M2[:, 0:SEG], M2[:, SEG:2 * SEG])
                    tt(M[:], M[:], M2[:, 0:SEG])
                    # V = 1 - Wn for the neg slots (on ACT: Copy(-x+1))
                    V = sp.tile([P, NEG], F32, tag="V")
                    nc.scalar.activation(out=V[:], in_=M[:, CTX:SEG],
                                         func=AF.Copy, scale=-1.0, bias=1.0)
                    nc.vector.tensor_reduce(
                        out=PRODS[:, 2 * t:2 * t + 1], in_=M[:, 0:CTX],
                        axis=AX.X, op=OP.mult)
                    nc.vector.tensor_reduce(
                        out=PRODS[:, 2 * t + 1:2 * t + 2], in_=V[:],
                        axis=AX.X, op=OP.mult)

            # ---- loss tail: ln(prod of all factors), summed over partitions
            PR = pp.tile([P, 1], F32)
            nc.vector.tensor_reduce(out=PR[:], in_=PRODS[:], axis=AX.X,
                                    op=OP.mult)
            nc.vector.tensor_scalar_max(out=PR[:], in0=PR[:], scalar1=TINY)
            LNP = pp.tile([P, 1], F32)
            nc.scalar.activation(out=LNP[:], in_=PR[:], func=AF.Ln)
            ps = psp.tile([1, 1], F32)
            nc.tensor.matmul(out=ps[:], lhsT=LNP[:], rhs=ones[:],
                             start=True, stop=True)
            res = pp.tile([1, 1], F32)
            nc.scalar.copy(res[:], ps[:])
            nc.sync.dma_start(out=out_d[:, :], in_=res[:])

    nc.compile()
    _CACHE[key] = nc
    return nc


def _prep_inputs(word_ids, context_ids, neg_ids,
                 emb_weight, global_emb_weight, disamb_weight):
    import ml_dtypes
    BF = ml_dtypes.bfloat16
    word_ids = np.asarray(word_ids).astype(np.int32).reshape(BATCH)
    context_ids = np.asarray(context_ids).astype(np.int32).reshape(BATCH, CTX)
    neg_ids = np.asarray(neg_ids).astype(np.int32).reshape(BATCH, NEG)
    packed = np.empty((VOCAB, RowLen), dtype=BF)
    packed[:, 0:D] = np.asarray(
        global_emb_weight, dtype=np.float32).astype(BF).reshape(VOCAB, D)
    packed[:, EMB_OFF:DIS_OFF] = np.asarray(
        emb_weight, dtype=np.float32).astype(BF).reshape(VOCAB, NUM_SENSE * D)
    packed[:, DIS_OFF:RowLen] = np.asarray(
        disamb_weight, dtype=np.float32).astype(BF).reshape(VOCAB, NUM_SENSE * D)
    # idx columns: 0..9 ctx, 10 word, 11..15 neg
    idx_all = np.concatenate(
        [context_ids, word_ids[:, None], neg_ids], axis=1).astype(np.int32)
    in_maps = []
    for c in range(N_CORES):
        sl = slice(c * PER_CORE, (c + 1) * PER_CORE)
        in_maps.append({
            "packed": packed,
            "idx": np.ascontiguousarray(idx_all[sl]),
        })
    return in_maps


def kernel(word_ids, context_ids, context_masks, neg_ids,
           emb_weight, global_emb_weight, disamb_weight):
    from concourse import bass_utils
    nc = _build_bass()
    in_maps = _prep_inputs(word_ids, context_ids, neg_ids,
                           emb_weight, global_emb_weight, disamb_weight)
    res = bass_utils.run_bass_kernel_spmd(nc, in_maps, core_ids=list(range(N_CORES)))
    total = 0.0
    for r in res.results:
        total += float(np.asarray(r["out"]).reshape(-1)[0])
    loss = -total / float(BATCH * CTX)
    return np.array(loss, dtype=np.float32)
